# revision 21
# baseline (speedup 1.0000x reference)
"""Trainium2 Bass kernel for masked (sparse) multi-head attention.

Reference (per batch): qkv = x @ w_qkv.T; q *= D**-0.5; s = q@k.T per head;
e = exp(s - max) * ap  (ap = key policy, self-attend always allowed);
attn = (e + eps/N) / (sum_m e + eps); y = (attn @ v) @ w_proj.T + b_proj.

Sharding: data parallel, batch b -> core b (B == n_cores == 8). No
collectives; weights are replicated.

Design notes (cost model: matmul time = out-free-size x cycles/row; the
output partition dim, contraction depth and weight loads are free):
  - host pre-transposes x / weights and PERMUTES tokens kept-first, so
    scores/exp/P@v run over only mk = ceil(kept/128) key chunks; dropped
    keys contribute only their diagonal self-term; rows un-permuted on host.
  - x, w_qkv, w_proj stream in as fp16.
  - scores are computed transposed, ST[m, n]: the key mask is a free
    per-partition ACT bias (exp(s + logmask[m])); exp is split across the
    ACT and GPSIMD engines (both run InstActivation) so neither gates PE.
  - P@v runs in NATURAL layout: out[token, 65] per (head, key-chunk,
    token-chunk) with P[keys, token-chunk] as the (free) stationary operand
    -- 65 free rows instead of 1024, ~2x cheaper than transposed P@v. Each
    head's v block carries a ones column at col 64, so the softmax
    denominator lands as a per-token PSUM column.
  - four (token-chunk) groups pack into one PSUM bank with a single
    start/stop accumulation group; a strided reciprocal gives per-token
    1/denom, and the PSUM-drain copy IS the normalization: one
    tensor_scalar_mul per (head, token-chunk) with a per-partition scalar.
  - the diagonal self-term is a diag(gm) stationary matmul per chunk >= jd
    (gm = (1-pol) * exp(q.k)); its ones column adds the self-term to the
    denominator for free.
  - normalized o[token, c] fp16 is transposed back to oT[c, token] with PE
    transpose matmuls (8 per 128-col band share one fp16 PSUM bank), so the
    projection contracts full 128-row c-chunks: 6 matmuls per token chunk.
  - output projection y[token, :] accumulates over 6 c-chunks; bias +
    un-permute on host.
"""

import sys

import numpy as np

sys.path.insert(0, "/opt/trn_rl_repo")

from contextlib import ExitStack

import concourse.bass as bass
import concourse.tile as tile
from concourse import mybir
from concourse.bacc import Bacc
from concourse.bass import BassScalarEngine

F32 = mybir.dt.float32
F32R = mybir.dt.float32r
BF16 = mybir.dt.bfloat16
FP16 = mybir.dt.float16
FP8 = mybir.dt.float8e4
AF = mybir.ActivationFunctionType
WS = 16.0              # host scales w_qkv by 16 into fp8's sweet spot;
                       # compensated in the qkv PSUM-drain copies

B, N, C, H = 8, 1024, 768, 12
D = C // H            # 64
SCALE = D ** -0.5
EPS = 1e-6
CH = C // 128          # 6 c-chunks (2 heads each)
NJ = N // 128          # 8 token chunks
MJ = N // 128
NEG = -10000.0         # exp(s + NEG) == 0.0 in fp32 for any realistic s
W = D + 1              # per-head v block: 64 cols of v + ones column


def build_nc(mk: int, jd: int) -> bass.Bass:
    """mk = chunks holding all kept tokens; jd = first chunk with any
    dropped token (diag machinery only needed for chunks >= jd)."""
    nc = Bacc()

    # fp8 DoubleRow operands: [j, p, i, m] = src[j*256 + i*128 + p, m]
    x8 = [nc.declare_dram_parameter(f"x8_{t}", [CH // 2, 128, 2, N], FP8,
                                    isOutput=False) for t in range(2)]
    w8 = {g: [nc.declare_dram_parameter(f"w8{g}_{t}", [CH // 2, 128, 2, C],
                                        FP8, isOutput=False)
              for t in range(2)] for g in "qkv"}
    wp8 = [nc.declare_dram_parameter(f"wp8_{t}", [CH // 2, 128, 2, C], FP8,
                                     isOutput=False) for t in range(2)]
    cpackA = nc.declare_dram_parameter("cpackA", [128, 2 * MJ], F32,
                                       isOutput=False)
    cpackB = nc.declare_dram_parameter("cpackB", [128, CH * H], F32R,
                                       isOutput=False)
    bpack = nc.declare_dram_parameter("bpack", [128, 128], BF16,
                                      isOutput=False)
    y = nc.declare_dram_parameter("y", [N, C], F32, isOutput=True)

    def pool_exp(out, in_, bias):
        BassScalarEngine.activation(nc.gpsimd, out, in_, AF.Exp, bias=bias)

    with ExitStack() as ctx:
        tc = ctx.enter_context(tile.TileContext(nc))

        consts = ctx.enter_context(tc.tile_pool(name="consts", bufs=1))
        qk_pool = ctx.enter_context(tc.tile_pool(name="qk", bufs=1))
        v_pool = ctx.enter_context(tc.tile_pool(name="v", bufs=1))
        o_pool = ctx.enter_context(tc.tile_pool(name="o", bufs=1))

        # ---- constants --------------------------------------------------
        cpa_sb = consts.tile([128, 2 * MJ], F32, tag="cpa", name="cpa")
        cpb_sb = consts.tile([128, CH * H], F32R, tag="cpb", name="cpb")
        id_sb = consts.tile([128, 128], BF16, tag="bp2", name="bp2")
        lm_sb = cpa_sb[:, 0:MJ]
        omp_sb = cpa_sb[:, MJ:2 * MJ]
        eh_sb = cpb_sb
        gm_sb = consts.tile([128, MJ, H], F32, tag="gm", name="gm")

        # persistent activation tiles
        qT = [qk_pool.tile([128, N], F32R, tag=f"qT{cc}", name=f"qT{cc}")
              for cc in range(CH)]
        kT = [qk_pool.tile([128, N], F32R, tag=f"kT{cc}", name=f"kT{cc}")
              for cc in range(CH)]
        nv = max(mk, MJ if jd < MJ else mk)   # chunks needing v (keys + diag)
        v65 = [v_pool.tile([128, H, W], FP16, tag=f"v{j}", name=f"v{j}")
               for j in range(nv)]
        # o bands: [128 tokens, 8 chunks, 128 c-cols] per c-chunk (2 heads)
        o_band = [o_pool.tile([128, NJ, 128], FP16, tag=f"ob{cc}",
                              name=f"ob{cc}") for cc in range(CH)]
        # oT in fp8 DoubleRow pairs: [c-pair][128, 2, tokens], value+residual
        oT0 = [o_pool.tile([128, 2, N], FP8, tag=f"oT0{jp}", name=f"oT0{jp}")
               for jp in range(CH // 2)]
        oT1 = [o_pool.tile([128, 2, N], FP8, tag=f"oT1{jp}", name=f"oT1{jp}")
               for jp in range(CH // 2)]

        # ================= phase 1: QKV =================================
        pp1 = ctx.enter_context(tc.tile_pool(name="psum", bufs=2, space="PSUM"))
        ph1 = ctx.enter_context(tc.tile_pool(name="ph1", bufs=1))
        gp = ctx.enter_context(tc.tile_pool(name="gmcsv", bufs=1))
        if True:
            # DMA order: x0 + q-columns of w0 first (q term-1 matmuls start
            # earliest), then residuals, then k / v columns, consts.
            JP = CH // 2       # 3 double-row contraction pairs
            x_sb = [[ph1.tile([128, 2, N], FP8, tag=f"x{t}{j}",
                              name=f"x{t}{j}") for j in range(JP)]
                    for t in range(2)]
            w_sb = {g: [[ph1.tile([128, 2, C], FP8, tag=f"w{g}{t}{j}",
                                  name=f"w{g}{t}{j}") for j in range(JP)]
                        for t in range(2)] for g in "qkv"}
            for t in range(2):
                for j in range(JP):
                    nc.sync.dma_start(out=x_sb[t][j][:], in_=x8[t][j])
                    nc.gpsimd.dma_start(out=w_sb["q"][t][j][:],
                                        in_=w8["q"][t][j])
            for g in ("k", "v"):
                for t in range(2):
                    for j in range(JP):
                        deng = nc.sync if t == 0 else nc.gpsimd
                        deng.dma_start(out=w_sb[g][t][j][:], in_=w8[g][t][j])
                if g == "k":
                    nc.gpsimd.dma_start(out=cpa_sb[:], in_=cpackA[:, :])
                    nc.gpsimd.dma_start(out=cpb_sb[:], in_=cpackB[:, :])
                    nc.gpsimd.dma_start(out=id_sb[:], in_=bpack[:, :])

            # ones columns of v65 (written once; copies fill cols 0:64)
            for j in range(nv):
                eng = nc.vector if j % 2 == 0 else nc.gpsimd
                eng.memset(v65[j][:, :, D], 1.0 / 32.0)

            DR = mybir.MatmulPerfMode.DoubleRow
            TERMS = ((0, 0), (0, 1), (1, 0))   # (w term, x term)

            # qT / kT: out[o_chunk, n] = sum_c w8[c, o] * x8[c, n], fp8
            # DoubleRow, 256-wide slices, 3 terms x 3 k-pairs per slice
            for g, dst, scl in (("q", qT, SCALE / WS), ("k", kT, 1.0 / WS)):
                for cc in range(CH):
                    for nn in range(2):
                        ps = pp1.tile([128, 512], F32, tag="ps5", name="qkps",
                                      bufs=5)
                        i = 0
                        for s2 in range(2):
                            off = nn * 512 + s2 * 256
                            for wt, xt in TERMS:
                                for j in range(JP):
                                    nc.tensor.matmul(
                                        ps[:, s2 * 256:(s2 + 1) * 256],
                                        w_sb[g][wt][j][:, :, cc * 128:(cc + 1) * 128],
                                        x_sb[xt][j][:, :, off:off + 256],
                                        start=(i == 0), stop=(i == 17),
                                        perf_mode=DR)
                                    i += 1
                        sl = dst[cc][:, nn * 512:(nn + 1) * 512]
                        if g == "q":
                            nc.vector.tensor_scalar_mul(sl, ps[:], scl)
                        else:
                            nc.scalar.mul(sl, ps[:], scl)

            # v natural: out[n_chunk, o] = sum_c x8[c, n] * w8v[c, o]
            for jn in range(nv):
                for si, (sl0, sl1) in enumerate(((0, 512), (512, C))):
                    ps = pp1.tile([128, 512], F32, tag="ps5", name="vpsum",
                                  bufs=5)
                    ns2 = (sl1 - sl0) // 256
                    i = 0
                    for s2 in range(ns2):
                        off = sl0 + s2 * 256
                        for wt, xt in TERMS:
                            for j in range(JP):
                                nc.tensor.matmul(
                                    ps[:, s2 * 256:(s2 + 1) * 256],
                                    x_sb[xt][j][:, :, jn * 128:(jn + 1) * 128],
                                    w_sb["v"][wt][j][:, :, off:off + 256],
                                    start=(i == 0), stop=(i == ns2 * 9 - 1),
                                    perf_mode=DR)
                                i += 1
                    h0, h1 = sl0 // D, sl1 // D
                    ps3 = ps[:, 0:sl1 - sl0].rearrange("p (h d) -> p h d",
                                                       h=h1 - h0)
                    eng = nc.vector if (jn + si) % 2 == 0 else nc.gpsimd
                    eng.tensor_scalar_mul(v65[jn][:, h0:h1, 0:D], ps3,
                                          1.0 / WS)

        def emit_gm():
            # gm = (1-pol) * exp(q.k) for chunks >= jd (diag self-term)
            prod = []
            for cc in range(CH):
                pr = gp.tile([128, N - jd * 128], F32R, tag=f"prod{cc}")
                eng = nc.gpsimd if cc % 2 == 0 else nc.vector
                eng.tensor_mul(pr[:], qT[cc][:, jd * 128:],
                               kT[cc][:, jd * 128:])
                prod.append(pr)
            for jm in range(jd, MJ):
                gps = pp1.tile([128, 512], F32, tag="pv", name="gmp")
                for cc in range(CH):
                    nc.tensor.matmul(
                        gps[:, 0:H],
                        prod[cc][:, (jm - jd) * 128:(jm - jd + 1) * 128],
                        eh_sb[:, cc * H:(cc + 1) * H],
                        start=(cc == 0), stop=(cc == CH - 1),
                    )
                nc.scalar.activation(gm_sb[:, jm, :], gps[:, 0:H], AF.Exp)
                nc.vector.tensor_scalar_mul(
                    gm_sb[:, jm, :], gm_sb[:, jm, :], omp_sb[:, jm:jm + 1])

        # ================= phase 2: attention ===========================
        HB = NJ // 2          # token chunks per psum bank-group
        with tc.tile_pool(name="wpp", bufs=1) as wpp, \
             tc.tile_pool(name="att", bufs=2) as ap_pool, \
             tc.tile_pool(name="diagp", bufs=4) as dg_pool, \
             tc.tile_pool(name="recp", bufs=2) as rec_pool:
          # early w_proj load (overlaps with attention compute)
          wp_sb = [[], []]
          for t in range(2):
              for jp in range(CH // 2):
                  wt = wpp.tile([128, 2, C], FP8, tag=f"wp{t}{jp}",
                                name=f"wp{t}{jp}")
                  nc.gpsimd.dma_start(out=wt[:], in_=wp8[t][jp])
                  wp_sb[t].append(wt)

          def emit_S_exp(h):
              cc, off = divmod(h, 2)
              off *= D
              P = []
              for jm in range(mk):
                  Sh = []
                  for nn in range(2):
                      S = pp1.tile([128, 512], F32, tag="ps5", name="S", bufs=5)
                      nc.tensor.matmul(
                          S[:],
                          kT[cc][off:off + D, jm * 128:(jm + 1) * 128],
                          qT[cc][off:off + D, nn * 512:(nn + 1) * 512],
                          start=True, stop=True)
                      Sh.append(S)
                  # exp halves drained in parallel on ACT + Pool
                  Pt = ap_pool.tile([128, N], FP16, tag=f"P{jm}", name="P")
                  a = jm % 2                           # ACT's half
                  nc.scalar.activation(Pt[:, a * 512:(a + 1) * 512], Sh[a][:],
                                       AF.Exp, bias=lm_sb[:, jm:jm + 1])
                  pool_exp(Pt[:, (1 - a) * 512:(2 - a) * 512], Sh[1 - a][:],
                           lm_sb[:, jm:jm + 1])
                  P.append(Pt)
              return P

          def emit_pv(h, P):
              cc, hh = divmod(h, 2)
              rec = rec_pool.tile([128, NJ], F32, tag="rec", name="rec")
              for half in range(2):
                  t0 = half * HB
                  pv = pp1.tile([128, 512], F32, tag="pv", name="pv")
                  pv4 = pv[:, 0:HB * W].rearrange("p (a b) -> p a b", a=HB)
                  # diag factors needed this half
                  dgs = {}
                  for t in range(t0, t0 + HB):
                      if t >= jd:
                          dg = dg_pool.tile([128, 128], BF16, tag="dg",
                                            name="dg")
                          nc.vector.tensor_scalar_mul(dg[:], id_sb[:],
                                                      gm_sb[:, t, h:h + 1])
                          dgs[t] = dg
                  n_mm = mk * HB + len(dgs)
                  i = 0
                  for jm in range(mk):
                      for ti in range(HB):
                          nc.tensor.matmul(
                              pv4[:, ti, :],
                              P[jm][:, (t0 + ti) * 128:(t0 + ti + 1) * 128],
                              v65[jm][:, h, :],
                              start=(i == 0), stop=(i == n_mm - 1))
                          i += 1
                  for t, dg in dgs.items():
                      nc.tensor.matmul(
                          pv4[:, t - t0, :], dg[:], v65[t][:, h, :],
                          start=False, stop=(i == n_mm - 1))
                      i += 1
                  with nc.allow_low_precision(reason="softmax denom"):
                      nc.vector.reciprocal(rec[:, t0:t0 + HB], pv4[:, :, D])
                  for ti in range(HB):
                      t = t0 + ti
                      eng = nc.vector if ti % 2 == 0 else nc.gpsimd
                      eng.tensor_scalar_mul(
                          o_band[cc][:, t, hh * D:(hh + 1) * D],
                          pv4[:, ti, 0:D], rec[:, t:t + 1])

          def emit_tp(cc, tail=False):
              tp = pp1.tile([128, NJ, 128], FP16, tag="tp", name="tp", bufs=1)
              for t in range(NJ):
                  nc.tensor.matmul(
                      tp[:, t, :], o_band[cc][:, t, :], id_sb[:],
                      start=(t == 0), stop=(t == NJ - 1), is_transpose=True)
              jp, i = divmod(cc, 2)
              rot = ((nc.scalar, nc.gpsimd, nc.scalar, nc.vector) if tail
                     else (nc.vector, nc.gpsimd, nc.vector, nc.scalar))
              for t in range(NJ):
                  o0 = oT0[jp][:, i, t * 128:(t + 1) * 128]
                  o1 = oT1[jp][:, i, t * 128:(t + 1) * 128]
                  eng = rot[t % 4]
                  if eng is nc.scalar:
                      eng.copy(o0, tp[:, t, :])
                  else:
                      eng.tensor_copy(o0, tp[:, t, :])
                  eng2 = (nc.gpsimd, nc.vector, nc.gpsimd, nc.vector)[t % 4]
                  eng2.tensor_sub(o1, tp[:, t, :], o0)

          prevP = None
          for h in range(H):
              P = emit_S_exp(h)
              if h == 0:
                  emit_gm()
              if prevP is not None:
                  emit_pv(h - 1, prevP)
                  if h >= 2 and h % 2 == 0:
                      emit_tp((h - 2) // 2)
              prevP = P
          emit_pv(H - 1, prevP)
          emit_tp((H - 2) // 2, tail=True)
          emit_tp((H - 1) // 2, tail=True)

          # ============= phase 3: output projection ====================
          with tc.tile_pool(name="ysb", bufs=2) as yp:
              YTERMS = ((0, 0), (0, 1), (1, 0))   # (o term, w term)
              for i in range(NJ):
                  ysb = yp.tile([128, C], F32, tag="ysb", name="ysb")
                  for si, (sl0, sl1) in enumerate(((0, 512), (512, C))):
                      yps = pp1.tile([128, 512], F32, tag="ps5", name="yps",
                                     bufs=5)
                      ns2 = (sl1 - sl0) // 256
                      k = 0
                      for s2 in range(ns2):
                          off = sl0 + s2 * 256
                          for jp in range(CH // 2):
                              for ot, wt in YTERMS:
                                  osrc = (oT0, oT1)[ot]
                                  nc.tensor.matmul(
                                      yps[:, s2 * 256:(s2 + 1) * 256],
                                      osrc[jp][:, :, i * 128:(i + 1) * 128],
                                      wp_sb[wt][jp][:, :, off:off + 256],
                                      start=(k == 0), stop=(k == ns2 * 9 - 1),
                                      perf_mode=DR)
                                  k += 1
                      eng = nc.scalar if si == 0 else nc.vector
                      if si == 0:
                          eng.mul(ysb[:, sl0:sl1], yps[:, 0:sl1 - sl0],
                                  1.0 / 256.0)
                      else:
                          eng.tensor_scalar_mul(ysb[:, sl0:sl1],
                                                yps[:, 0:sl1 - sl0],
                                                1.0 / 256.0)
                      deng = nc.sync if si == 0 else nc.gpsimd
                      deng.dma_start(out=y[i * 128:(i + 1) * 128, sl0:sl1],
                                     in_=ysb[:, sl0:sl1])

    nc.finalize()
    return nc


_NC_CACHE = {}


def _get_nc(mk: int = MJ, jd: int = 0):
    if (mk, jd) not in _NC_CACHE:
        _NC_CACHE[(mk, jd)] = build_nc(mk, jd)
    return _NC_CACHE[(mk, jd)]


def _to_bf16(a):
    import ml_dtypes
    return np.asarray(a, np.float32).astype(ml_dtypes.bfloat16)


def _host_inputs(x, policy, w_qkv, w_proj, b_proj):
    """Shard + permute (kept tokens first) + layout transforms.

    Returns (in_maps, perms, mk, jd)."""
    import ml_dtypes
    E4 = ml_dtypes.float8_e4m3

    def dr_split(a):
        """[C, M] f32 -> (val, residual) fp8 pair in DoubleRow layout
        [C//256, 128, 2, M]."""
        a0 = a.astype(E4)
        a1 = (a - a0.astype(np.float32)).astype(E4)
        out = []
        for q in (a0, a1):
            out.append(np.ascontiguousarray(
                q.reshape(CH // 2, 2, 128, a.shape[1]).transpose(0, 2, 1, 3)))
        return out

    wqkvT = np.ascontiguousarray(
        np.asarray(w_qkv, np.float32).T) * np.float32(WS)          # [C, 3C]
    w8 = {}
    for gi, g in enumerate("qkv"):
        w8[g] = dr_split(wqkvT[:, gi * C:(gi + 1) * C])
    wp8 = dr_split(np.ascontiguousarray(
        np.asarray(w_proj, np.float32).T) * np.float32(8.0))

    E = np.zeros((C, H), np.float32)
    for c in range(C):
        E[c, c // D] = 1.0
    Ehead = np.ascontiguousarray(
        E.reshape(CH, 128, H).transpose(1, 0, 2).reshape(128, CH * H))
    bpack = _to_bf16(np.eye(128, dtype=np.float32))
    cpackB = np.ascontiguousarray(Ehead)

    in_maps = []
    perms = []
    mk = 1
    jd = MJ - 1
    for b in range(B):
        pol = np.asarray(policy[b], np.float32).reshape(N)
        kept = np.nonzero(pol > 0.5)[0]
        drop = np.nonzero(pol <= 0.5)[0]
        perm = np.concatenate([kept, drop])
        perms.append(perm)
        mk = max(mk, (len(kept) + 127) // 128)
        jd = min(jd, len(kept) // 128)

        xb = np.asarray(x[b], np.float32)[perm, :]          # permuted tokens
        x8 = dr_split(np.ascontiguousarray(xb.T))           # [C, N] fp8 pair
        polp = pol[perm]
        lm = np.where(polp > 0.5, 0.0, NEG).astype(np.float32)
        lm = np.ascontiguousarray(lm.reshape(MJ, 128).T)    # [128, MJ]
        om = np.ascontiguousarray((1.0 - polp).reshape(MJ, 128).T)
        cpackA = np.ascontiguousarray(np.concatenate(
            [lm, om.astype(np.float32)], axis=1))
        in_maps.append({
            "x8_0": x8[0], "x8_1": x8[1],
            "w8q_0": w8["q"][0], "w8q_1": w8["q"][1],
            "w8k_0": w8["k"][0], "w8k_1": w8["k"][1],
            "w8v_0": w8["v"][0], "w8v_1": w8["v"][1],
            "wp8_0": wp8[0], "wp8_1": wp8[1],
            "cpackA": cpackA, "cpackB": cpackB, "bpack": bpack,
        })
    return in_maps, perms, mk, jd


def kernel(x, policy, w_qkv, w_proj, b_proj):
    from concourse.bass_utils import run_bass_kernel_spmd

    x = np.asarray(x, np.float32)
    policy = np.asarray(policy, np.float32)
    w_qkv = np.asarray(w_qkv, np.float32)
    w_proj = np.asarray(w_proj, np.float32)
    b_proj = np.asarray(b_proj, np.float32)
    in_maps, perms, mk, jd = _host_inputs(x, policy, w_qkv, w_proj, b_proj)
    nc = _get_nc(mk, jd)
    res = run_bass_kernel_spmd(nc, in_maps, list(range(B)))
    out = np.empty((B, N, C), np.float32)
    bp = np.asarray(b_proj, np.float32).reshape(1, C)
    for b in range(B):
        out[b][perms[b]] = res.results[b]["y"] + bp
    return out
